# revision 4
# baseline (speedup 1.0000x reference)
"""Channel-attention (nn_ChannelAttentionModule) Trainium2 kernel.

Math (per batch b):
    X = x[b]  [C, N]  with C=512, N=64*64=4096
    q = Wq X + bq ; k = Wk X + bk ; v = Wv X + bv
    L = q k^T                       [C, C]
    out = softmax(L, -1) v + X      [C, N]

Key restructure: L = Wq G Wk^T + bq (Wk S + N bk)^T_outer + (Wq S) bk^T_outer
  where G = X X^T (Gram, symmetric) and S = X 1 (row sums).
This moves all precision-critical work into G (computed with a bf16
hi/lo split: G = Xh Xh^T + M + M^T, M = Xh Xl^T) plus two tiny fp32
512^3 matmuls, leaving the v-path in fast fp16.

Sharding: pure data-parallel, one batch per NeuronCore (B=8, 8 cores).
"""

import numpy as np
import ml_dtypes

import concourse.bass as bass
import concourse.mybir as mybir
import concourse.tile as tile
from concourse import bacc
from concourse.bass_utils import run_bass_kernel_spmd

F32 = mybir.dt.float32
BF16 = mybir.dt.bfloat16
F16 = mybir.dt.float16
AX = mybir.AxisListType.X
EXP = mybir.ActivationFunctionType.Exp

B = 8
C = 512
HW = 64 * 64  # N spatial
P = 128
CH = C // P  # 4 chunks of channels
NCH = HW // P  # 32 chunks of spatial (G pass)
NT = HW // 512  # 8 tiles of spatial (v / out pass)


def _body(tc, nc, io):
    xth, xtl, x16, x32 = io["xth"], io["xtl"], io["x16"], io["x32"]
    wqt, wkt, wvt = io["wqt"], io["wkt"], io["wvt"]
    bqr, bkr, nbkr, bvc = io["bqr"], io["bkr"], io["nbkr"], io["bvc"]
    id32, id16, out = io["id32"], io["id16"], io["out"]

    ps = tc.alloc_tile_pool(name="ps", bufs=1, space="PSUM")
    sb = tc.alloc_tile_pool(name="sb", bufs=1)
    st = tc.alloc_tile_pool(name="st", bufs=3)
    sx = tc.alloc_tile_pool(name="sx", bufs=2)
    so = tc.alloc_tile_pool(name="so", bufs=2)

    # ---- constants / weights in ----
    wqt_sb = [sb.tile([P, C], F32, name=f"wqt{i}", tag=f"wqt{i}") for i in range(CH)]
    wkt_sb = [sb.tile([P, C], F32, name=f"wkt{i}", tag=f"wkt{i}") for i in range(CH)]
    wvt_sb = [sb.tile([P, C], F16, name=f"wvt{i}", tag=f"wvt{i}") for i in range(CH)]
    for i in range(CH):
        nc.sync.dma_start(wqt_sb[i], wqt[i * P : (i + 1) * P, :])
        nc.sync.dma_start(wkt_sb[i], wkt[i * P : (i + 1) * P, :])
        nc.sync.dma_start(wvt_sb[i], wvt[i * P : (i + 1) * P, :])
    id32_sb = sb.tile([P, P], F32, name="id32sb", tag="id32sb")
    nc.sync.dma_start(id32_sb, id32)
    id16_sb = sb.tile([P, P], F16, name="id16sb", tag="id16sb")
    nc.sync.dma_start(id16_sb, id16)
    nbkr_sb = sb.tile([1, C], F32, name="nbkrsb", tag="nbkrsb")
    nc.sync.dma_start(nbkr_sb, nbkr)
    bvc_sb = [sb.tile([P, 1], F32, name=f"bvc{i}", tag=f"bvc{i}") for i in range(CH)]
    for i in range(CH):
        nc.sync.dma_start(bvc_sb[i], bvc[i * P : (i + 1) * P, :])

    x32_sb = [sb.tile([P, HW], F32, name=f"x32_{i}", tag=f"x32_{i}") for i in range(CH)]
    for i in range(CH):
        nc.sync.dma_start(x32_sb[i], x32[i * P : (i + 1) * P, :])

    # ---- S = X @ 1 (row sums), during G pass (DVE) ----
    s_col = [sb.tile([P, 1], F32, name=f"s{i}", tag=f"s{i}") for i in range(CH)]
    for i in range(CH):
        nc.vector.reduce_sum(s_col[i], x32_sb[i], axis=AX)

    # ---- G pass: G_hh and M = Xh Xl^T accumulate over 32 spatial chunks ----
    g_ps = [ps.tile([P, C], F32, name=f"gps{i}", tag=f"pa{i}") for i in range(CH)]
    m_ps = [ps.tile([P, C], F32, name=f"mps{i}", tag=f"pb{i}") for i in range(CH)]
    for n in range(NCH):
        ah = st.tile([P, C], BF16, name="ah", tag="ah")
        nc.sync.dma_start(ah, xth[n * P : (n + 1) * P, :])
        al = st.tile([P, C], BF16, name="al", tag="al")
        nc.sync.dma_start(al, xtl[n * P : (n + 1) * P, :])
        first, last = n == 0, n == NCH - 1
        for c in range(CH):
            lhs = ah[:, c * P : (c + 1) * P]
            nc.tensor.matmul(g_ps[c], lhsT=lhs, rhs=ah, start=first, stop=last)
            nc.tensor.matmul(m_ps[c], lhsT=lhs, rhs=al, start=first, stop=last)

    # ---- G = Ghh + M + M^T (fp32, symmetric) ----
    m_sb = [sb.tile([P, C], F32, name=f"msb{i}", tag=f"msb{i}") for i in range(CH)]
    g_sb = [sb.tile([P, C], F32, name=f"gsb{i}", tag=f"gsb{i}") for i in range(CH)]
    for c in range(CH):
        nc.vector.tensor_copy(m_sb[c], m_ps[c])
        nc.vector.tensor_add(g_sb[c], g_ps[c], m_sb[c])
    mt_ps = [ps.tile([P, C], F32, name=f"mtps{j}", tag=f"pb{j}") for j in range(CH)]
    for j in range(CH):
        for i in range(CH):
            nc.tensor.transpose(
                mt_ps[j][:, i * P : (i + 1) * P],
                m_sb[i][:, j * P : (j + 1) * P],
                id32_sb,
            )
    for c in range(CH):
        nc.vector.tensor_add(g_sb[c], g_sb[c], mt_ps[c])

    # ---- T1 = G Wk^T (fp32) ----
    t1_ps = [ps.tile([P, C], F32, name=f"t1ps{i}", tag=f"pa{i}") for i in range(CH)]
    for e in range(CH):
        for f in range(CH):
            nc.tensor.matmul(
                t1_ps[e],
                lhsT=g_sb[f][:, e * P : (e + 1) * P],
                rhs=wkt_sb[f],
                start=f == 0,
                stop=f == CH - 1,
            )
    t1_sb = [sb.tile([P, C], F32, name=f"t1sb{i}", tag=f"t1sb{i}") for i in range(CH)]
    for e in range(CH):
        nc.vector.tensor_copy(t1_sb[e], t1_ps[e])

    # ---- u1 = (Wq S)^T, u2 = (Wk S)^T as [1, C] rows (via K=128 matmuls) ----
    u1_ps = ps.tile([1, C], F32, name="u1ps", tag="pb0")
    u2_ps = ps.tile([1, C], F32, name="u2ps", tag="pb1")
    for e in range(CH):
        nc.tensor.matmul(
            u1_ps, lhsT=s_col[e], rhs=wqt_sb[e], start=e == 0, stop=e == CH - 1
        )
    for e in range(CH):
        nc.tensor.matmul(
            u2_ps, lhsT=s_col[e], rhs=wkt_sb[e], start=e == 0, stop=e == CH - 1
        )
    bqr_sb = sb.tile([1, C], F32, name="bqr_sb", tag="bqr_sb")
    nc.sync.dma_start(bqr_sb, bqr)
    bkr_sb = sb.tile([1, C], F32, name="bkr_sb", tag="bkr_sb")
    nc.sync.dma_start(bkr_sb, bkr)
    u1_sb = sb.tile([1, C], F32, name="u1_sb", tag="u1_sb")
    nc.vector.tensor_copy(u1_sb, u1_ps)
    r0_sb = sb.tile([1, C], F32, name="r0_sb", tag="r0_sb")
    nc.vector.tensor_add(r0_sb, u2_ps, nbkr_sb)

    # ---- logits = Wq T1 + rank-1 bias terms (fp32, accumulated in PSUM) ----
    l_ps = [ps.tile([P, C], F32, name=f"lps{i}", tag=f"pb{i}") for i in range(CH)]
    for c in range(CH):
        for e in range(CH):
            nc.tensor.matmul(
                l_ps[c],
                lhsT=wqt_sb[e][:, c * P : (c + 1) * P],
                rhs=t1_sb[e],
                start=e == 0,
                stop=False,
            )
        nc.tensor.matmul(
            l_ps[c],
            lhsT=bqr_sb[:, c * P : (c + 1) * P],
            rhs=r0_sb,
            start=False,
            stop=False,
        )
        nc.tensor.matmul(
            l_ps[c],
            lhsT=u1_sb[:, c * P : (c + 1) * P],
            rhs=bkr_sb,
            start=False,
            stop=True,
        )

    # ---- v = Wv X + bv (fp16 path), fills PE while softmax runs ----
    v_sb = [sb.tile([P, HW], F16, name=f"vsb{i}", tag=f"vsb{i}") for i in range(CH)]
    for nt in range(NT):
        xv = [None] * CH
        for c in range(CH):
            xv[c] = sx.tile([P, 512], F16, name=f"xv{c}", tag=f"xv{c}")
            nc.sync.dma_start(xv[c], x16[c * P : (c + 1) * P, nt * 512 : (nt + 1) * 512])
        for o in range(CH):
            v_ps = ps.tile([P, 512], F32, name=f"vps{o}", tag=f"pa{o}")
            for c in range(CH):
                nc.tensor.matmul(
                    v_ps,
                    lhsT=wvt_sb[c][:, o * P : (o + 1) * P],
                    rhs=xv[c],
                    start=c == 0,
                    stop=c == CH - 1,
                )
            nc.vector.tensor_scalar_add(
                v_sb[o][:, nt * 512 : (nt + 1) * 512], v_ps, bvc_sb[o]
            )

    # ---- softmax over free dim (rows of L) ----
    w16_sb = [sb.tile([P, C], F16, name=f"w16_{i}", tag=f"w16_{i}") for i in range(CH)]
    for c in range(CH):
        negmx = sb.tile([P, 1], F32, name=f"negmx{c}", tag=f"negmx{c}")
        nc.vector.reduce_max(negmx, l_ps[c], axis=AX, negate=True)
        e_sb = sb.tile([P, C], F32, name=f"esb{c}", tag=f"esb{c}")
        ssum = sb.tile([P, 1], F32, name=f"ssum{c}", tag=f"ssum{c}")
        nc.scalar.activation(e_sb, l_ps[c], EXP, bias=negmx, scale=1.0, accum_out=ssum)
        rcp = sb.tile([P, 1], F32, name=f"rcp{c}", tag=f"rcp{c}")
        nc.vector.reciprocal(rcp, ssum)
        nc.vector.tensor_scalar_mul(w16_sb[c], e_sb, rcp)

    # ---- transpose softmax weights (fp16, PE transpose) ----
    wt_sb = [sb.tile([P, C], F16, name=f"wtsb{j}", tag=f"wtsb{j}") for j in range(CH)]
    for j in range(CH):
        wt_ps = ps.tile([P, C], F16, name=f"wtps{j}", tag=f"pb{j}")
        for i in range(CH):
            nc.tensor.transpose(
                wt_ps[:, i * P : (i + 1) * P],
                w16_sb[i][:, j * P : (j + 1) * P],
                id16_sb,
            )
        nc.vector.tensor_copy(wt_sb[j], wt_ps)

    # ---- out = w v + X (fp16 matmul, fp32 residual) ----
    for nt in range(NT):
        for c in range(CH):
            o_ps = ps.tile([P, 512], F32, name=f"ops{c}", tag=f"pa{c}")
            for d in range(CH):
                nc.tensor.matmul(
                    o_ps,
                    lhsT=wt_sb[d][:, c * P : (c + 1) * P],
                    rhs=v_sb[d][:, nt * 512 : (nt + 1) * 512],
                    start=d == 0,
                    stop=d == CH - 1,
                )
            o_sb = so.tile([P, 512], F32, name=f"osb{c}", tag=f"osb{c}")
            nc.vector.tensor_add(
                o_sb, o_ps, x32_sb[c][:, nt * 512 : (nt + 1) * 512]
            )
            nc.sync.dma_start(
                out[c * P : (c + 1) * P, nt * 512 : (nt + 1) * 512], o_sb
            )

    for pool in (so, sx, st, sb, ps):
        pool.release()


def _build_nc():
    nc = bacc.Bacc(
        "TRN2",
        target_bir_lowering=False,
        debug=False,
        num_devices=B,
        enable_asserts=False,
    )
    io = {}
    dt = nc.dram_tensor
    io["xth"] = dt("xth", (HW, C), BF16, kind="ExternalInput").ap()
    io["xtl"] = dt("xtl", (HW, C), BF16, kind="ExternalInput").ap()
    io["x16"] = dt("x16", (C, HW), F16, kind="ExternalInput").ap()
    io["x32"] = dt("x32", (C, HW), F32, kind="ExternalInput").ap()
    io["wqt"] = dt("wqt", (C, C), F32, kind="ExternalInput").ap()
    io["wkt"] = dt("wkt", (C, C), F32, kind="ExternalInput").ap()
    io["wvt"] = dt("wvt", (C, C), F16, kind="ExternalInput").ap()
    io["bqr"] = dt("bqr", (1, C), F32, kind="ExternalInput").ap()
    io["bkr"] = dt("bkr", (1, C), F32, kind="ExternalInput").ap()
    io["nbkr"] = dt("nbkr", (1, C), F32, kind="ExternalInput").ap()
    io["bvc"] = dt("bvc", (C, 1), F32, kind="ExternalInput").ap()
    io["id32"] = dt("id32", (P, P), F32, kind="ExternalInput").ap()
    io["id16"] = dt("id16", (P, P), F16, kind="ExternalInput").ap()
    io["out"] = dt("out", (C, HW), F32, kind="ExternalOutput").ap()
    with tile.TileContext(nc) as tc:
        _body(tc, nc, io)
    nc.compile()
    return nc


_NC_CACHE = None


def get_nc():
    global _NC_CACHE
    if _NC_CACHE is None:
        _NC_CACHE = _build_nc()
    return _NC_CACHE


def prep_in_maps(x, wq, bq, wk, bk, wv, bv):
    """Host-side input prep: reshape/transpose/dtype-split only."""
    x = np.asarray(x, dtype=np.float32)
    X = x.reshape(B, C, HW)
    XT = np.ascontiguousarray(X.transpose(0, 2, 1))
    xth = XT.astype(ml_dtypes.bfloat16)
    xtl = (XT - xth.astype(np.float32)).astype(ml_dtypes.bfloat16)
    x16 = X.astype(np.float16)
    wqt = np.ascontiguousarray(np.asarray(wq, np.float32).T)
    wkt = np.ascontiguousarray(np.asarray(wk, np.float32).T)
    wvt = np.ascontiguousarray(np.asarray(wv, np.float32).T).astype(np.float16)
    bqr = np.asarray(bq, np.float32).reshape(1, C)
    bkr = np.asarray(bk, np.float32).reshape(1, C)
    nbkr = (float(HW) * np.asarray(bk, np.float32)).reshape(1, C)
    bvc = np.asarray(bv, np.float32).reshape(C, 1)
    id32 = np.eye(P, dtype=np.float32)
    id16 = np.eye(P, dtype=np.float16)
    in_maps = []
    for b in range(B):
        in_maps.append(
            {
                "xth": xth[b],
                "xtl": xtl[b],
                "x16": np.ascontiguousarray(x16[b]),
                "x32": np.ascontiguousarray(X[b]),
                "wqt": wqt,
                "wkt": wkt,
                "wvt": wvt,
                "bqr": bqr,
                "bkr": bkr,
                "nbkr": nbkr,
                "bvc": bvc,
                "id32": id32,
                "id16": id16,
            }
        )
    return in_maps


def kernel(x, wq, bq, wk, bk, wv, bv):
    nc = get_nc()
    in_maps = prep_in_maps(x, wq, bq, wk, bk, wv, bv)
    res = run_bass_kernel_spmd(nc, in_maps, core_ids=list(range(B)))
    out = np.stack([res.results[b]["out"] for b in range(B)])
    return out.reshape(B, C, 64, 64).astype(np.float32)
